# revision 20
# baseline (speedup 1.0000x reference)
"""AttnBlock (GroupNorm + single-head self-attention + residual) on 8 trn2 cores.

Problem: x[8, 512, 2048]; per batch element:
    h = GroupNorm32(x) * gn_scale + gn_bias
    q/k/v = w{q,k,v} @ h + b  (1x1 conv == channel matmul), layout [L, C]
    w = softmax(q k^T / sqrt(C)); a = w v
    out = x + (wo @ a^T + bo)

Sharding: pure data-parallel over batch (B=8 == 8 cores), one batch element
per NeuronCore; weights replicated. No collectives.

Per-core layout strategy (C=512 -> 4 partition tiles, L=2048):
  - h, Q^T, K^T kept as [C-part, L-free]; Vl computed as [L-part, C-free]
  - S^T tile [k-part, q-free] = (K^T chunk).T @ Q^T  -> softmax denominator is
    a column sum == ones-matmul; no P transposes anywhere
  - A^T [C-part, q-free] = Vl.T @ P^T feeds the out-projection directly
  - softmax without max-subtraction (logits are ~N(0,1); |logit| < ~8)
"""

import sys
import numpy as np

if "/opt/trn_rl_repo" not in sys.path:
    sys.path.insert(0, "/opt/trn_rl_repo")

import concourse.bass as bass
import concourse.bacc as bacc
import concourse.mybir as mybir
from concourse import tile
from concourse import bass_isa
from concourse import library_config

FP32 = mybir.dt.float32
FP32R = mybir.dt.float32r

C = 512
L = 2048
G = 32
CPG = C // G  # 16 channels per group
EPS = 1e-5
NCT = C // 128  # 4 channel tiles
NLT = L // 128  # 16 L tiles
NQC = L // 512  # 4 q chunks
SCALE = float(np.float32(1.0) / np.sqrt(np.float32(C)))

# Matmul operand dtype. FP32R streams at 1 cycle/row (vs 4 for FP32) when
# the moving free dim is >= 256, at a small precision cost (~2e-5 rel err
# end-to-end vs ~9e-7 for FP32; measured on hardware). Every matmul input
# tile is produced by a compute instruction with an FP32R output dtype,
# which the BIR verifier requires (operands must be pre-rounded).
# Set MM_DT = FP32 for full-fp32 matmuls (~4.5x slower on the PE).
MM_DT = FP32R


def _r(ap):
    return ap


def build_kernel(nc, reps=1):
    x_d = nc.declare_dram_parameter("x", [C, L], FP32, isOutput=False)
    gns_d = nc.declare_dram_parameter("gn_scale", [C], FP32, isOutput=False)
    gnb_d = nc.declare_dram_parameter("gn_bias", [C], FP32, isOutput=False)
    wq_d = nc.declare_dram_parameter("wq", [C, C], FP32, isOutput=False)
    bq_d = nc.declare_dram_parameter("bq", [C], FP32, isOutput=False)
    wk_d = nc.declare_dram_parameter("wk", [C, C], FP32, isOutput=False)
    bk_d = nc.declare_dram_parameter("bk", [C], FP32, isOutput=False)
    wv_d = nc.declare_dram_parameter("wv", [C, C], FP32, isOutput=False)
    bv_d = nc.declare_dram_parameter("bv", [C], FP32, isOutput=False)
    wo_d = nc.declare_dram_parameter("wo", [C, C], FP32, isOutput=False)
    bo_d = nc.declare_dram_parameter("bo", [C], FP32, isOutput=False)
    out_d = nc.declare_dram_parameter("out", [C, L], FP32, isOutput=True)

    with tile.TileContext(nc) as tc:
        for _ in range(reps):
            _body(nc, tc, x_d, gns_d, gnb_d, wq_d, bq_d, wk_d, bk_d,
                  wv_d, bv_d, wo_d, bo_d, out_d)
    return nc


def _body(nc, tc, x_d, gns_d, gnb_d, wq_d, bq_d, wk_d, bk_d,
          wv_d, bv_d, wo_d, bo_d, out_d):
    from contextlib import ExitStack

    Id = mybir.ActivationFunctionType.Identity
    Exp = mybir.ActivationFunctionType.Exp
    Sqrt = mybir.ActivationFunctionType.Sqrt
    Square = mybir.ActivationFunctionType.Square
    add = mybir.AluOpType.add
    mult = mybir.AluOpType.mult
    sub = mybir.AluOpType.subtract

    with ExitStack() as ctx:
        consts = ctx.enter_context(tc.tile_pool(name="consts", bufs=1))
        vecs = ctx.enter_context(tc.tile_pool(name="vecs", bufs=1))
        qt_pool = ctx.enter_context(tc.tile_pool(name="qt", bufs=NCT))
        kt_pool = ctx.enter_context(tc.tile_pool(name="kt", bufs=NCT))
        vl_pool = ctx.enter_context(tc.tile_pool(name="vl", bufs=NLT))
        wot_pool = ctx.enter_context(tc.tile_pool(name="wot", bufs=NCT))

        # ---- constants ----
        ident = consts.tile([128, 128], FP32, tag="ident")
        nc.vector.memset(ident[:], 1.0)
        # keep where (i - p) == 0
        nc.gpsimd.affine_select(ident[:], ident[:], [[1, 128]],
                                mybir.AluOpType.is_equal, 0.0,
                                base=0, channel_multiplier=-1)
        # group indicator Ind[p, g] = 1 iff p//16 == g   (iota = p - 16 g)
        ind = consts.tile([128, G // 4], FP32, tag="ind")  # [128, 8]
        nc.vector.memset(ind[:], 1.0)
        nc.gpsimd.affine_select(ind[:], ind[:], [[-CPG, G // 4]],
                                mybir.AluOpType.is_ge, 0.0,
                                base=0, channel_multiplier=1)
        nc.gpsimd.affine_select(ind[:], ind[:], [[CPG, G // 4]],
                                mybir.AluOpType.is_ge, 0.0,
                                base=CPG - 1, channel_multiplier=-1)
        # J[g, p] = 1 iff p//16 == g  (iota = p - 16 g)
        jmat = consts.tile([G // 4, 128], FP32, tag="jmat")  # [8, 128]
        nc.vector.memset(jmat[:], 1.0)
        nc.gpsimd.affine_select(jmat[:], jmat[:], [[1, 128]],
                                mybir.AluOpType.is_ge, 0.0,
                                base=0, channel_multiplier=-CPG)
        nc.gpsimd.affine_select(jmat[:], jmat[:], [[-1, 128]],
                                mybir.AluOpType.is_ge, 0.0,
                                base=CPG - 1, channel_multiplier=CPG)

        # GPSIMD ucode library providing partition_all_reduce
        nc.gpsimd.load_library(library_config.attn)

        # ---- per-partition vectors: [512] -> 4 x [128,1] ----
        def load_vec(dram, name):
            t = vecs.tile([128, NCT], FP32, tag=name, name=name + "_sb")
            for i in range(NCT):
                nc.sync.dma_start(out=t[:, i:i + 1],
                                  in_=dram[128 * i:128 * (i + 1)].unsqueeze(-1))
            return t


        qt_t, kt_t, vl_t = [], [], []
        wo_T = []

        with ExitStack() as setup_ctx:
            wsb = setup_ctx.enter_context(tc.tile_pool(name="wsb", bufs=2))
            xp = setup_ctx.enter_context(tc.tile_pool(name="xp", bufs=2))
            hp = setup_ctx.enter_context(tc.tile_pool(name="hp", bufs=NCT))
            gn_sb = setup_ctx.enter_context(tc.tile_pool(name="gnsb", bufs=1))
            wqT_pool = setup_ctx.enter_context(tc.tile_pool(name="wqT", bufs=NCT))
            wkT_pool = setup_ctx.enter_context(tc.tile_pool(name="wkT", bufs=NCT))
            wvT_pool = setup_ctx.enter_context(tc.tile_pool(name="wvT", bufs=NCT))
            ps_t = setup_ctx.enter_context(
                tc.tile_pool(name="ps_t", bufs=2, space="PSUM"))
            ps_gn = setup_ctx.enter_context(
                tc.tile_pool(name="ps_gn", bufs=1, space="PSUM"))
            ps_p = setup_ctx.enter_context(
                tc.tile_pool(name="ps_p", bufs=4, space="PSUM"))

            # ---- transpose the four weights (PE transpose) ----
            for w_d, nm, pool, dst in ((wq_d, "wq", wqT_pool, None),
                                       (wk_d, "wk", wkT_pool, None),
                                       (wv_d, "wv", wvT_pool, None),
                                       (wo_d, "wo", wot_pool, wo_T)):
                wT_tiles = [pool.tile([128, C], MM_DT,
                                      tag=pool.name, name=f"{nm}T{t}")
                            for t in range(NCT)]
                for u in range(NCT):
                    w_t = wsb.tile([128, C], FP32, tag="wsb", name=f"w{nm}{u}")
                    nc.sync.dma_start(out=w_t[:],
                                      in_=w_d[128 * u:128 * (u + 1), :])
                    for t in range(NCT):
                        tp = ps_t.tile([128, 128], FP32, tag="tp", name="tp")
                        nc.tensor.transpose(tp[:],
                                            w_t[:, 128 * t:128 * (t + 1)],
                                            ident[:])
                        nc.vector.tensor_copy(
                            wT_tiles[t][:, 128 * u:128 * (u + 1)], tp[:])
                if nm == "wq":
                    wq_T = wT_tiles
                elif nm == "wk":
                    wk_T = wT_tiles
                elif nm == "wv":
                    wv_T = wT_tiles
                else:
                    wo_T.extend(wT_tiles)

            # small per-partition vectors (issued after the big loads so
            # they don't head-block the DMA queues)
            gns_t = load_vec(gns_d, "gns")
            gnb_t = load_vec(gnb_d, "gnb")
            bq_t = load_vec(bq_d, "bq")
            bk_t = load_vec(bk_d, "bk")
            bo_t = load_vec(bo_d, "bo")

            # ---- load x, compute GN stats ----
            ht = [hp.tile([128, L], MM_DT, tag="hp", name=f"h{t}")
                  for t in range(NCT)]
            stats = gn_sb.tile([128, 2 * NCT], FP32, tag="stats")
            xtiles = {}
            for t in range(NCT):
                x_t = xp.tile([128, L], FP32, tag="xp", name=f"x{t}")
                xtiles[t] = x_t
                nc.sync.dma_start(out=x_t[:], in_=x_d[128 * t:128 * (t + 1), :])
                # per-partition sum and sum of squares (square output is
                # dumped into the h tile, later overwritten by the GN apply)
                nc.vector.tensor_reduce(stats[:, 2 * t:2 * t + 1], x_t[:],
                                        mybir.AxisListType.X, add)
                nc.scalar.activation(ht[t][:], x_t[:], Square,
                                     accum_out=stats[:, 2 * t + 1:2 * t + 2])

            # ---- finish GN stats: group reduce + broadcast ----
            inv_n = float(1.0 / (CPG * L))
            gsum_ps = ps_gn.tile([G // 4, 2 * NCT], FP32, tag="gsum")
            nc.tensor.matmul(gsum_ps[:], ind[:], stats[:])
            # mr[:, 0:4] = mean, mr[:, 4:8] = rstd   per c-tile column
            mr = gn_sb.tile([G // 4, 2 * NCT], FP32, tag="mr")
            tmp8 = gn_sb.tile([G // 4, NCT], FP32, tag="tmp8")
            gview = gsum_ps.rearrange("p (c two) -> p c two", two=2)
            nc.vector.tensor_scalar_mul(mr[:, 0:NCT], gview[:, :, 0], inv_n)
            nc.vector.tensor_scalar_mul(tmp8[:], gview[:, :, 1], inv_n)
            # var = E[x^2] - mean^2 ; rstd = 1/sqrt(var + eps)
            var8 = gn_sb.tile([G // 4, NCT], FP32, tag="var8")
            nc.vector.tensor_tensor(var8[:], mr[:, 0:NCT], mr[:, 0:NCT], mult)
            nc.vector.tensor_tensor(var8[:], tmp8[:], var8[:], sub)
            std8 = gn_sb.tile([G // 4, NCT], FP32, tag="std8")
            eps8 = gn_sb.tile([G // 4, 1], FP32, tag="eps8")
            nc.vector.memset(eps8[:], EPS)
            nc.scalar.activation(std8[:], var8[:], Sqrt, bias=eps8[:])
            nc.vector.reciprocal(mr[:, NCT:2 * NCT], std8[:])

            # broadcast mean/rstd to per-partition, apply GN affine.
            # xp (bufs=2) still holds x2, x3 after the stats pass, so apply
            # those first and only re-DMA x0, x1.
            mr_v = mr.rearrange("p (h f) -> p h f", h=2)
            APPLY_ORDER = [2, 3, 0, 1]
            for t in APPLY_ORDER:
                bc = ps_t.tile([128, 2], FP32, tag="tp", name=f"bc{t}")
                nc.tensor.matmul(bc[:], jmat[:], mr_v[:, :, t])
                a_t = gn_sb.tile([128, 1], FP32, tag=f"a{t}", name=f"a{t}")
                b_t = gn_sb.tile([128, 1], FP32, tag=f"b{t}", name=f"b{t}")
                nc.vector.tensor_tensor(a_t[:], bc[:, 1:2], gns_t[:, t:t + 1],
                                        mult)
                # b = gn_bias - mean * a
                nc.vector.tensor_tensor(b_t[:], bc[:, 0:1], a_t[:], mult)
                nc.vector.tensor_tensor(b_t[:], gnb_t[:, t:t + 1], b_t[:], sub)
                if t in (2, 3):
                    x_src = xtiles[t]
                else:
                    x_src = xp.tile([128, L], FP32, tag="xp", name=f"xb{t}")
                    nc.sync.dma_start(out=x_src[:],
                                      in_=x_d[128 * t:128 * (t + 1), :])
                nc.scalar.activation(ht[t][:], x_src[:], Id,
                                     bias=b_t[:], scale=a_t[:])

            # ---- bv broadcast [128, C] via replicated DMA ----
            bvb = gn_sb.tile([128, C], FP32, tag="bvb")
            nc.sync.dma_start(out=bvb[:],
                              in_=bv_d[:].unsqueeze(0).partition_broadcast(128))

            # ---- projections ----
            for dst_list, wT_l, bvec, pool, nmo in (
                    (qt_t, None, bq_t, qt_pool, "q"),
                    (kt_t, None, bk_t, kt_pool, "k")):
                wT_l = wq_T if nmo == "q" else wk_T
                for t in range(NCT):
                    dtile = pool.tile([128, L], MM_DT, tag=pool.name,
                                      name=f"{nmo}T{t}")
                    for lc in range(NQC):
                        pp = ps_p.tile([128, 512], FP32, tag="pp", name="pp")
                        for j, ci in enumerate([2, 3, 0, 1]):
                            nc.tensor.matmul(
                                pp[:],
                                _r(wT_l[ci][:, 128 * t:128 * (t + 1)]),
                                _r(ht[ci][:, 512 * lc:512 * (lc + 1)]),
                                start=(j == 0), stop=(j == NCT - 1))
                        nc.scalar.activation(dtile[:, 512 * lc:512 * (lc + 1)],
                                             pp[:], Id, bias=bvec[:, t:t + 1])
                    dst_list.append(dtile)

            for lt in range(NLT):
                vtile = vl_pool.tile([128, C], MM_DT, tag="vl",
                                     name=f"vl{lt}")
                pp = ps_p.tile([128, 512], FP32, tag="pp", name="pp")
                for j, ci in enumerate([2, 3, 0, 1]):
                    nc.tensor.matmul(
                        pp[:],
                        _r(ht[ci][:, 128 * lt:128 * (lt + 1)]),
                        _r(wv_T[ci][:]),
                        start=(j == 0), stop=(j == NCT - 1))
                nc.vector.tensor_tensor(vtile[:], pp[:], bvb[:], add)
                vl_t.append(vtile)

        # ---- attention ----
        with ExitStack() as att_ctx:
            pt_pool = att_ctx.enter_context(tc.tile_pool(name="pt", bufs=3))
            dinv_pool = att_ctx.enter_context(tc.tile_pool(name="dinv", bufs=2))
            asb_pool = att_ctx.enter_context(tc.tile_pool(name="asb", bufs=8))
            osb_pool = att_ctx.enter_context(tc.tile_pool(name="osb", bufs=3))
            xr_pool = att_ctx.enter_context(tc.tile_pool(name="xr", bufs=3))
            dacc_pool = att_ctx.enter_context(tc.tile_pool(name="dacc", bufs=2))
            ps_s = att_ctx.enter_context(
                tc.tile_pool(name="ps_s", bufs=3, space="PSUM"))
            ps_a = att_ctx.enter_context(
                tc.tile_pool(name="ps_a", bufs=NCT, space="PSUM"))
            ps_o = att_ctx.enter_context(
                tc.tile_pool(name="ps_o", bufs=1, space="PSUM"))

            def s_block(qc_i, kt_i):
                s_tile = ps_s.tile([128, 512], FP32, tag="s",
                                   name=f"s{qc_i}_{kt_i}")
                for ci in range(NCT):
                    nc.tensor.matmul(
                        s_tile[:],
                        _r(kt_t[ci][:, 128 * kt_i:128 * (kt_i + 1)]),
                        _r(qt_t[ci][:, 512 * qc_i:512 * qc_i + 512]),
                        start=(ci == 0), stop=(ci == NCT - 1))
                return s_tile

            s_next = s_block(0, 0)
            for qc in range(NQC):
                q0 = 512 * qc
                a_ps = [ps_a.tile([128, 512], FP32, tag="a", name=f"a_ps{cc}")
                        for cc in range(NCT)]
                dacc = dacc_pool.tile([128, 512], FP32, tag="dacc",
                                      name="dacc")

                s_cur = s_next
                for kt_i in range(NLT):
                    p_t = pt_pool.tile([128, 512], MM_DT, tag="pt",
                                       name="p_t")
                    nc.scalar.activation(p_t[:], s_cur[:], Exp, scale=SCALE)
                    if kt_i + 1 < NLT:
                        s_cur = s_block(qc, kt_i + 1)
                    elif qc + 1 < NQC:
                        # prefetch the next q-chunk's first S block so the PE
                        # has work while DVE finishes this chunk's division
                        s_next = s_block(qc + 1, 0)
                    first = kt_i == 0
                    last = kt_i == NLT - 1
                    for cc in range(NCT):
                        nc.tensor.matmul(
                            a_ps[cc][:],
                            _r(vl_t[kt_i][:, 128 * cc:128 * (cc + 1)]),
                            _r(p_t[:]),
                            start=first, stop=last, skip_group_check=True)
                    # softmax denominator: accumulate exp tiles on DVE (fp32)
                    if first:
                        nc.vector.tensor_copy(dacc[:], p_t[:])
                    else:
                        nc.vector.tensor_tensor(dacc[:], dacc[:], p_t[:], add)

                # cross-partition sum on the (otherwise idle) GPSIMD engine
                dsum = dinv_pool.tile([128, 512], FP32, tag="dsum",
                                      name="dsum")
                nc.gpsimd.partition_all_reduce(dsum[:], dacc[:], 128,
                                               bass_isa.ReduceOp.add)
                dinv = dinv_pool.tile([128, 512], FP32, tag="dinv", name="dinv")
                dscr = dinv_pool.tile([128, 512], FP32, tag="dscr", name="dscr")
                nc.vector.reciprocal_approx_accurate(out=dinv[:], in_=dsum[:],
                                                     scratch=dscr[:])
                a_sb = []
                for cc in range(NCT):
                    t = asb_pool.tile([128, 512], MM_DT, tag="asb",
                                      name=f"asb{cc}")
                    nc.vector.tensor_tensor(t[:], a_ps[cc][:], dinv[:], mult)
                    a_sb.append(t)

                for ot in range(NCT):
                    o_ps = ps_o.tile([128, 512], FP32, tag="o", name="o_ps")
                    for cc in range(NCT):
                        nc.tensor.matmul(
                            o_ps[:],
                            _r(wo_T[cc][:, 128 * ot:128 * (ot + 1)]),
                            _r(a_sb[cc][:]),
                            start=(cc == 0), stop=(cc == NCT - 1))
                    xr = xr_pool.tile([128, 512], FP32, tag="xr", name="xr")
                    nc.sync.dma_start(
                        out=xr[:],
                        in_=x_d[128 * ot:128 * (ot + 1), q0:q0 + 512])
                    osb = osb_pool.tile([128, 512], FP32, tag="osb", name="osb")
                    nc.vector.scalar_tensor_tensor(
                        osb[:], o_ps[:], bo_t[:, ot:ot + 1], xr[:],
                        op0=add, op1=add)
                    nc.sync.dma_start(
                        out=out_d[128 * ot:128 * (ot + 1), q0:q0 + 512],
                        in_=osb[:])


def make_nc():
    return bacc.Bacc("TRN2", target_bir_lowering=False, debug=False)


_NC_CACHE = []


def kernel(**inputs):
    from concourse.bass_utils import run_bass_kernel_spmd

    x = np.ascontiguousarray(inputs["x"], dtype=np.float32)
    B = x.shape[0]
    assert B == 8, f"kernel is built for B=8 (one batch element per core), got {B}"
    shared = {}
    for name in ("gn_scale", "gn_bias", "wq", "bq", "wk", "bk",
                 "wv", "bv", "wo", "bo"):
        shared[name] = np.ascontiguousarray(inputs[name], dtype=np.float32)

    if not _NC_CACHE:
        nc = make_nc()
        build_kernel(nc)
        nc.compile()
        _NC_CACHE.append(nc)
    nc = _NC_CACHE[0]

    core_ids = list(range(B))
    in_maps = [dict(shared, x=x[i]) for i in range(B)]
    res = run_bass_kernel_spmd(nc, in_maps, core_ids)
    out = np.stack([res.results[i]["out"] for i in range(B)], axis=0)
    return out.astype(np.float32)


if __name__ == "__main__":
    rng = np.random.default_rng(0)
    demo = {
        "x": rng.standard_normal((8, C, L), dtype=np.float32),
        "gn_scale": np.ones(C, np.float32),
        "gn_bias": np.zeros(C, np.float32),
    }
    for w, b in (("wq", "bq"), ("wk", "bk"), ("wv", "bv"), ("wo", "bo")):
        demo[w] = rng.standard_normal((C, C), dtype=np.float32) / np.sqrt(C)
        demo[b] = np.zeros(C, np.float32)
    out = kernel(**demo)
    print(out.shape, out.dtype)


# revision 21
# speedup vs baseline: 1.1003x; 1.1003x over previous
"""AttnBlock (GroupNorm + single-head self-attention + residual) on 8 trn2 cores.

Problem: x[8, 512, 2048]; per batch element:
    h = GroupNorm32(x) * gn_scale + gn_bias
    q/k/v = w{q,k,v} @ h + b  (1x1 conv == channel matmul), layout [L, C]
    w = softmax(q k^T / sqrt(C)); a = w v
    out = x + (wo @ a^T + bo)

Sharding: pure data-parallel over batch (B=8 == 8 cores), one batch element
per NeuronCore; weights replicated. No collectives.

Per-core layout strategy (C=512 -> 4 partition tiles, L=2048):
  - h, Q^T, K^T kept as [C-part, L-free]; Vl computed as [L-part, C-free]
  - S^T tile [k-part, q-free] = (K^T chunk).T @ Q^T  -> softmax denominator is
    a column sum == ones-matmul; no P transposes anywhere
  - A^T [C-part, q-free] = Vl.T @ P^T feeds the out-projection directly
  - softmax without max-subtraction (logits are ~N(0,1); |logit| < ~8)
"""

import sys
import numpy as np

if "/opt/trn_rl_repo" not in sys.path:
    sys.path.insert(0, "/opt/trn_rl_repo")

import concourse.bass as bass
import concourse.bacc as bacc
import concourse.mybir as mybir
from concourse import tile
from concourse import bass_isa
from concourse import library_config

FP32 = mybir.dt.float32
FP32R = mybir.dt.float32r

C = 512
L = 2048
G = 32
CPG = C // G  # 16 channels per group
EPS = 1e-5
NCT = C // 128  # 4 channel tiles
NLT = L // 128  # 16 L tiles
NQC = L // 512  # 4 q chunks
SCALE = float(np.float32(1.0) / np.sqrt(np.float32(C)))

# Matmul operand dtype. FP32R streams at 1 cycle/row (vs 4 for FP32) when
# the moving free dim is >= 256, at a small precision cost (~2e-5 rel err
# end-to-end vs ~9e-7 for FP32; measured on hardware). Every matmul input
# tile is produced by a compute instruction with an FP32R output dtype,
# which the BIR verifier requires (operands must be pre-rounded).
# Set MM_DT = FP32 for full-fp32 matmuls (~4.5x slower on the PE).
MM_DT = FP32R


def _r(ap):
    return ap


def build_kernel(nc, reps=1):
    x_d = nc.declare_dram_parameter("x", [C, L], FP32, isOutput=False)
    gns_d = nc.declare_dram_parameter("gn_scale", [C], FP32, isOutput=False)
    gnb_d = nc.declare_dram_parameter("gn_bias", [C], FP32, isOutput=False)
    wq_d = nc.declare_dram_parameter("wq", [C, C], FP32, isOutput=False)
    bq_d = nc.declare_dram_parameter("bq", [C], FP32, isOutput=False)
    wk_d = nc.declare_dram_parameter("wk", [C, C], FP32, isOutput=False)
    bk_d = nc.declare_dram_parameter("bk", [C], FP32, isOutput=False)
    wv_d = nc.declare_dram_parameter("wv", [C, C], FP32, isOutput=False)
    bv_d = nc.declare_dram_parameter("bv", [C], FP32, isOutput=False)
    wo_d = nc.declare_dram_parameter("wo", [C, C], FP32, isOutput=False)
    bo_d = nc.declare_dram_parameter("bo", [C], FP32, isOutput=False)
    out_d = nc.declare_dram_parameter("out", [C, L], FP32, isOutput=True)

    with tile.TileContext(nc) as tc:
        for _ in range(reps):
            _body(nc, tc, x_d, gns_d, gnb_d, wq_d, bq_d, wk_d, bk_d,
                  wv_d, bv_d, wo_d, bo_d, out_d)
    return nc


def _body(nc, tc, x_d, gns_d, gnb_d, wq_d, bq_d, wk_d, bk_d,
          wv_d, bv_d, wo_d, bo_d, out_d):
    from contextlib import ExitStack

    Id = mybir.ActivationFunctionType.Identity
    Exp = mybir.ActivationFunctionType.Exp
    Sqrt = mybir.ActivationFunctionType.Sqrt
    Square = mybir.ActivationFunctionType.Square
    add = mybir.AluOpType.add
    mult = mybir.AluOpType.mult
    sub = mybir.AluOpType.subtract

    with ExitStack() as ctx:
        consts = ctx.enter_context(tc.tile_pool(name="consts", bufs=1))
        vecs = ctx.enter_context(tc.tile_pool(name="vecs", bufs=1))
        qt_pool = ctx.enter_context(tc.tile_pool(name="qt", bufs=NCT))
        kt_pool = ctx.enter_context(tc.tile_pool(name="kt", bufs=NCT))
        vl_pool = ctx.enter_context(tc.tile_pool(name="vl", bufs=NLT))
        wot_pool = ctx.enter_context(tc.tile_pool(name="wot", bufs=NCT))

        # ---- constants ----
        onesf = consts.tile([128, 128], FP32, tag="onesf")
        nc.vector.memset(onesf[:], 1.0)
        ones = consts.tile([128, 128], MM_DT, tag="ones")
        nc.vector.tensor_copy(ones[:], onesf[:])
        ident = consts.tile([128, 128], FP32, tag="ident")
        nc.vector.memset(ident[:], 1.0)
        # keep where (i - p) == 0
        nc.gpsimd.affine_select(ident[:], ident[:], [[1, 128]],
                                mybir.AluOpType.is_equal, 0.0,
                                base=0, channel_multiplier=-1)
        # group indicator Ind[p, g] = 1 iff p//16 == g   (iota = p - 16 g)
        ind = consts.tile([128, G // 4], FP32, tag="ind")  # [128, 8]
        nc.vector.memset(ind[:], 1.0)
        nc.gpsimd.affine_select(ind[:], ind[:], [[-CPG, G // 4]],
                                mybir.AluOpType.is_ge, 0.0,
                                base=0, channel_multiplier=1)
        nc.gpsimd.affine_select(ind[:], ind[:], [[CPG, G // 4]],
                                mybir.AluOpType.is_ge, 0.0,
                                base=CPG - 1, channel_multiplier=-1)
        # J[g, p] = 1 iff p//16 == g  (iota = p - 16 g)
        jmat = consts.tile([G // 4, 128], FP32, tag="jmat")  # [8, 128]
        nc.vector.memset(jmat[:], 1.0)
        nc.gpsimd.affine_select(jmat[:], jmat[:], [[1, 128]],
                                mybir.AluOpType.is_ge, 0.0,
                                base=0, channel_multiplier=-CPG)
        nc.gpsimd.affine_select(jmat[:], jmat[:], [[-1, 128]],
                                mybir.AluOpType.is_ge, 0.0,
                                base=CPG - 1, channel_multiplier=CPG)

        # ---- per-partition vectors: [512] -> 4 x [128,1] ----
        def load_vec(dram, name):
            t = vecs.tile([128, NCT], FP32, tag=name, name=name + "_sb")
            for i in range(NCT):
                nc.sync.dma_start(out=t[:, i:i + 1],
                                  in_=dram[128 * i:128 * (i + 1)].unsqueeze(-1))
            return t


        qt_t, kt_t, vl_t = [], [], []
        wo_T = []

        with ExitStack() as setup_ctx:
            wsb = setup_ctx.enter_context(tc.tile_pool(name="wsb", bufs=2))
            xp = setup_ctx.enter_context(tc.tile_pool(name="xp", bufs=2))
            hp = setup_ctx.enter_context(tc.tile_pool(name="hp", bufs=NCT))
            gn_sb = setup_ctx.enter_context(tc.tile_pool(name="gnsb", bufs=1))
            wqT_pool = setup_ctx.enter_context(tc.tile_pool(name="wqT", bufs=NCT))
            wkT_pool = setup_ctx.enter_context(tc.tile_pool(name="wkT", bufs=NCT))
            wvT_pool = setup_ctx.enter_context(tc.tile_pool(name="wvT", bufs=NCT))
            ps_t = setup_ctx.enter_context(
                tc.tile_pool(name="ps_t", bufs=2, space="PSUM"))
            ps_gn = setup_ctx.enter_context(
                tc.tile_pool(name="ps_gn", bufs=1, space="PSUM"))
            ps_p = setup_ctx.enter_context(
                tc.tile_pool(name="ps_p", bufs=4, space="PSUM"))

            # ---- transpose the four weights (PE transpose) ----
            for w_d, nm, pool, dst in ((wq_d, "wq", wqT_pool, None),
                                       (wk_d, "wk", wkT_pool, None),
                                       (wv_d, "wv", wvT_pool, None),
                                       (wo_d, "wo", wot_pool, wo_T)):
                wT_tiles = [pool.tile([128, C], MM_DT,
                                      tag=pool.name, name=f"{nm}T{t}")
                            for t in range(NCT)]
                for u in range(NCT):
                    w_t = wsb.tile([128, C], FP32, tag="wsb", name=f"w{nm}{u}")
                    nc.sync.dma_start(out=w_t[:],
                                      in_=w_d[128 * u:128 * (u + 1), :])
                    for t in range(NCT):
                        tp = ps_t.tile([128, 128], FP32, tag="tp", name="tp")
                        nc.tensor.transpose(tp[:],
                                            w_t[:, 128 * t:128 * (t + 1)],
                                            ident[:])
                        nc.vector.tensor_copy(
                            wT_tiles[t][:, 128 * u:128 * (u + 1)], tp[:])
                if nm == "wq":
                    wq_T = wT_tiles
                elif nm == "wk":
                    wk_T = wT_tiles
                elif nm == "wv":
                    wv_T = wT_tiles
                else:
                    wo_T.extend(wT_tiles)

            # small per-partition vectors (issued after the big loads so
            # they don't head-block the DMA queues)
            gns_t = load_vec(gns_d, "gns")
            gnb_t = load_vec(gnb_d, "gnb")
            bq_t = load_vec(bq_d, "bq")
            bk_t = load_vec(bk_d, "bk")
            bo_t = load_vec(bo_d, "bo")

            # ---- load x, compute GN stats ----
            ht = [hp.tile([128, L], MM_DT, tag="hp", name=f"h{t}")
                  for t in range(NCT)]
            stats = gn_sb.tile([128, 2 * NCT], FP32, tag="stats")
            xtiles = {}
            for t in range(NCT):
                x_t = xp.tile([128, L], FP32, tag="xp", name=f"x{t}")
                xtiles[t] = x_t
                nc.sync.dma_start(out=x_t[:], in_=x_d[128 * t:128 * (t + 1), :])
                # per-partition sum and sum of squares (square output is
                # dumped into the h tile, later overwritten by the GN apply)
                nc.vector.tensor_reduce(stats[:, 2 * t:2 * t + 1], x_t[:],
                                        mybir.AxisListType.X, add)
                nc.scalar.activation(ht[t][:], x_t[:], Square,
                                     accum_out=stats[:, 2 * t + 1:2 * t + 2])

            # ---- finish GN stats: group reduce + broadcast ----
            inv_n = float(1.0 / (CPG * L))
            gsum_ps = ps_gn.tile([G // 4, 2 * NCT], FP32, tag="gsum")
            nc.tensor.matmul(gsum_ps[:], ind[:], stats[:])
            # mr[:, 0:4] = mean, mr[:, 4:8] = rstd   per c-tile column
            mr = gn_sb.tile([G // 4, 2 * NCT], FP32, tag="mr")
            tmp8 = gn_sb.tile([G // 4, NCT], FP32, tag="tmp8")
            gview = gsum_ps.rearrange("p (c two) -> p c two", two=2)
            nc.vector.tensor_scalar_mul(mr[:, 0:NCT], gview[:, :, 0], inv_n)
            nc.vector.tensor_scalar_mul(tmp8[:], gview[:, :, 1], inv_n)
            # var = E[x^2] - mean^2 ; rstd = 1/sqrt(var + eps)
            var8 = gn_sb.tile([G // 4, NCT], FP32, tag="var8")
            nc.vector.tensor_tensor(var8[:], mr[:, 0:NCT], mr[:, 0:NCT], mult)
            nc.vector.tensor_tensor(var8[:], tmp8[:], var8[:], sub)
            std8 = gn_sb.tile([G // 4, NCT], FP32, tag="std8")
            eps8 = gn_sb.tile([G // 4, 1], FP32, tag="eps8")
            nc.vector.memset(eps8[:], EPS)
            nc.scalar.activation(std8[:], var8[:], Sqrt, bias=eps8[:])
            nc.vector.reciprocal(mr[:, NCT:2 * NCT], std8[:])

            # broadcast mean/rstd to per-partition, apply GN affine.
            # xp (bufs=2) still holds x2, x3 after the stats pass, so apply
            # those first and only re-DMA x0, x1.
            mr_v = mr.rearrange("p (h f) -> p h f", h=2)
            APPLY_ORDER = [2, 3, 0, 1]
            for t in APPLY_ORDER:
                bc = ps_t.tile([128, 2], FP32, tag="tp", name=f"bc{t}")
                nc.tensor.matmul(bc[:], jmat[:], mr_v[:, :, t])
                a_t = gn_sb.tile([128, 1], FP32, tag=f"a{t}", name=f"a{t}")
                b_t = gn_sb.tile([128, 1], FP32, tag=f"b{t}", name=f"b{t}")
                nc.vector.tensor_tensor(a_t[:], bc[:, 1:2], gns_t[:, t:t + 1],
                                        mult)
                # b = gn_bias - mean * a
                nc.vector.tensor_tensor(b_t[:], bc[:, 0:1], a_t[:], mult)
                nc.vector.tensor_tensor(b_t[:], gnb_t[:, t:t + 1], b_t[:], sub)
                if t in (2, 3):
                    x_src = xtiles[t]
                else:
                    x_src = xp.tile([128, L], FP32, tag="xp", name=f"xb{t}")
                    nc.sync.dma_start(out=x_src[:],
                                      in_=x_d[128 * t:128 * (t + 1), :])
                nc.scalar.activation(ht[t][:], x_src[:], Id,
                                     bias=b_t[:], scale=a_t[:])

            # ---- bv broadcast [128, C] via replicated DMA ----
            bvb = gn_sb.tile([128, C], FP32, tag="bvb")
            nc.sync.dma_start(out=bvb[:],
                              in_=bv_d[:].unsqueeze(0).partition_broadcast(128))

            # ---- projections ----
            for dst_list, wT_l, bvec, pool, nmo in (
                    (qt_t, None, bq_t, qt_pool, "q"),
                    (kt_t, None, bk_t, kt_pool, "k")):
                wT_l = wq_T if nmo == "q" else wk_T
                for t in range(NCT):
                    dtile = pool.tile([128, L], MM_DT, tag=pool.name,
                                      name=f"{nmo}T{t}")
                    for lc in range(NQC):
                        pp = ps_p.tile([128, 512], FP32, tag="pp", name="pp")
                        for j, ci in enumerate([2, 3, 0, 1]):
                            nc.tensor.matmul(
                                pp[:],
                                _r(wT_l[ci][:, 128 * t:128 * (t + 1)]),
                                _r(ht[ci][:, 512 * lc:512 * (lc + 1)]),
                                start=(j == 0), stop=(j == NCT - 1))
                        nc.scalar.activation(dtile[:, 512 * lc:512 * (lc + 1)],
                                             pp[:], Id, bias=bvec[:, t:t + 1])
                    dst_list.append(dtile)

            for lt in range(NLT):
                vtile = vl_pool.tile([128, C], MM_DT, tag="vl",
                                     name=f"vl{lt}")
                pp = ps_p.tile([128, 512], FP32, tag="pp", name="pp")
                for j, ci in enumerate([2, 3, 0, 1]):
                    nc.tensor.matmul(
                        pp[:],
                        _r(ht[ci][:, 128 * lt:128 * (lt + 1)]),
                        _r(wv_T[ci][:]),
                        start=(j == 0), stop=(j == NCT - 1))
                nc.vector.tensor_tensor(vtile[:], pp[:], bvb[:], add)
                vl_t.append(vtile)

        # ---- attention ----
        with ExitStack() as att_ctx:
            pt_pool = att_ctx.enter_context(tc.tile_pool(name="pt", bufs=3))
            dinv_pool = att_ctx.enter_context(tc.tile_pool(name="dinv", bufs=2))
            asb_pool = att_ctx.enter_context(tc.tile_pool(name="asb", bufs=8))
            osb_pool = att_ctx.enter_context(tc.tile_pool(name="osb", bufs=3))
            xr_pool = att_ctx.enter_context(tc.tile_pool(name="xr", bufs=3))
            ps_s = att_ctx.enter_context(
                tc.tile_pool(name="ps_s", bufs=2, space="PSUM"))
            ps_a = att_ctx.enter_context(
                tc.tile_pool(name="ps_a", bufs=NCT, space="PSUM"))
            ps_d = att_ctx.enter_context(
                tc.tile_pool(name="ps_d", bufs=1, space="PSUM"))
            ps_o = att_ctx.enter_context(
                tc.tile_pool(name="ps_o", bufs=1, space="PSUM"))

            def s_block(qc_i, kt_i):
                s_tile = ps_s.tile([128, 512], FP32, tag="s",
                                   name=f"s{qc_i}_{kt_i}")
                for ci in range(NCT):
                    nc.tensor.matmul(
                        s_tile[:],
                        _r(kt_t[ci][:, 128 * kt_i:128 * (kt_i + 1)]),
                        _r(qt_t[ci][:, 512 * qc_i:512 * qc_i + 512]),
                        start=(ci == 0), stop=(ci == NCT - 1))
                return s_tile

            s_next = s_block(0, 0)
            for qc in range(NQC):
                q0 = 512 * qc
                a_ps = [ps_a.tile([128, 512], FP32, tag="a", name=f"a_ps{cc}")
                        for cc in range(NCT)]
                d_ps = ps_d.tile([128, 512], FP32, tag="d", name="d_ps")

                s_cur = s_next
                for kt_i in range(NLT):
                    p_t = pt_pool.tile([128, 512], MM_DT, tag="pt",
                                       name="p_t")
                    nc.scalar.activation(p_t[:], s_cur[:], Exp, scale=SCALE)
                    if kt_i + 1 < NLT:
                        s_cur = s_block(qc, kt_i + 1)
                    elif qc + 1 < NQC:
                        # prefetch the next q-chunk's first S block so the PE
                        # has work while DVE finishes this chunk's division
                        s_next = s_block(qc + 1, 0)
                    first = kt_i == 0
                    last = kt_i == NLT - 1
                    for cc in range(NCT):
                        nc.tensor.matmul(
                            a_ps[cc][:],
                            _r(vl_t[kt_i][:, 128 * cc:128 * (cc + 1)]),
                            _r(p_t[:]),
                            start=first, stop=last, skip_group_check=True)
                    # softmax denominator: ones-matmul accumulation (the
                    # result lands broadcast across all 128 partitions)
                    nc.tensor.matmul(d_ps[:], _r(ones[:]), _r(p_t[:]),
                                     start=first, stop=last,
                                     skip_group_check=True)

                dinv = dinv_pool.tile([128, 512], FP32, tag="dinv", name="dinv")
                dscr = dinv_pool.tile([128, 512], FP32, tag="dscr", name="dscr")
                nc.vector.reciprocal_approx_accurate(out=dinv[:], in_=d_ps[:],
                                                     scratch=dscr[:])
                a_sb = []
                for cc in range(NCT):
                    t = asb_pool.tile([128, 512], MM_DT, tag="asb",
                                      name=f"asb{cc}")
                    nc.vector.tensor_tensor(t[:], a_ps[cc][:], dinv[:], mult)
                    a_sb.append(t)

                for ot in range(NCT):
                    o_ps = ps_o.tile([128, 512], FP32, tag="o", name="o_ps")
                    for cc in range(NCT):
                        nc.tensor.matmul(
                            o_ps[:],
                            _r(wo_T[cc][:, 128 * ot:128 * (ot + 1)]),
                            _r(a_sb[cc][:]),
                            start=(cc == 0), stop=(cc == NCT - 1))
                    xr = xr_pool.tile([128, 512], FP32, tag="xr", name="xr")
                    nc.sync.dma_start(
                        out=xr[:],
                        in_=x_d[128 * ot:128 * (ot + 1), q0:q0 + 512])
                    osb = osb_pool.tile([128, 512], FP32, tag="osb", name="osb")
                    nc.vector.scalar_tensor_tensor(
                        osb[:], o_ps[:], bo_t[:, ot:ot + 1], xr[:],
                        op0=add, op1=add)
                    nc.sync.dma_start(
                        out=out_d[128 * ot:128 * (ot + 1), q0:q0 + 512],
                        in_=osb[:])


def make_nc():
    return bacc.Bacc("TRN2", target_bir_lowering=False, debug=False)


_NC_CACHE = []


def kernel(**inputs):
    from concourse.bass_utils import run_bass_kernel_spmd

    x = np.ascontiguousarray(inputs["x"], dtype=np.float32)
    B = x.shape[0]
    assert B == 8, f"kernel is built for B=8 (one batch element per core), got {B}"
    shared = {}
    for name in ("gn_scale", "gn_bias", "wq", "bq", "wk", "bk",
                 "wv", "bv", "wo", "bo"):
        shared[name] = np.ascontiguousarray(inputs[name], dtype=np.float32)

    if not _NC_CACHE:
        nc = make_nc()
        build_kernel(nc)
        nc.compile()
        _NC_CACHE.append(nc)
    nc = _NC_CACHE[0]

    core_ids = list(range(B))
    in_maps = [dict(shared, x=x[i]) for i in range(B)]
    res = run_bass_kernel_spmd(nc, in_maps, core_ids)
    out = np.stack([res.results[i]["out"] for i in range(B)], axis=0)
    return out.astype(np.float32)


if __name__ == "__main__":
    rng = np.random.default_rng(0)
    demo = {
        "x": rng.standard_normal((8, C, L), dtype=np.float32),
        "gn_scale": np.ones(C, np.float32),
        "gn_bias": np.zeros(C, np.float32),
    }
    for w, b in (("wq", "bq"), ("wk", "bk"), ("wv", "bv"), ("wo", "bo")):
        demo[w] = rng.standard_normal((C, C), dtype=np.float32) / np.sqrt(C)
        demo[b] = np.zeros(C, np.float32)
    out = kernel(**demo)
    print(out.shape, out.dtype)


# revision 22
# speedup vs baseline: 1.2000x; 1.0905x over previous
"""AttnBlock (GroupNorm + single-head self-attention + residual) on 8 trn2 cores.

Problem: x[8, 512, 2048]; per batch element:
    h = GroupNorm32(x) * gn_scale + gn_bias
    q/k/v = w{q,k,v} @ h + b  (1x1 conv == channel matmul), layout [L, C]
    w = softmax(q k^T / sqrt(C)); a = w v
    out = x + (wo @ a^T + bo)

Sharding: pure data-parallel over batch (B=8 == 8 cores), one batch element
per NeuronCore; weights replicated. No collectives.

Per-core layout strategy (C=512 -> 4 partition tiles, L=2048):
  - h, Q^T, K^T kept as [C-part, L-free]; Vl computed as [L-part, C-free]
  - S^T tile [k-part, q-free] = (K^T chunk).T @ Q^T  -> softmax denominator is
    a column sum == ones-matmul; no P transposes anywhere
  - A^T [C-part, q-free] = Vl.T @ P^T feeds the out-projection directly
  - softmax without max-subtraction (logits are ~N(0,1); |logit| < ~8)
"""

import sys
import numpy as np

if "/opt/trn_rl_repo" not in sys.path:
    sys.path.insert(0, "/opt/trn_rl_repo")

import concourse.bass as bass
import concourse.bacc as bacc
import concourse.mybir as mybir
from concourse import tile
from concourse import bass_isa
from concourse import library_config

FP32 = mybir.dt.float32
FP32R = mybir.dt.float32r

C = 512
L = 2048
G = 32
CPG = C // G  # 16 channels per group
EPS = 1e-5
NCT = C // 128  # 4 channel tiles
NLT = L // 128  # 16 L tiles
NQC = L // 512  # 4 q chunks
SCALE = float(np.float32(1.0) / np.sqrt(np.float32(C)))

# Matmul operand dtype. FP32R streams at 1 cycle/row (vs 4 for FP32) when
# the moving free dim is >= 256, at a small precision cost (~2e-5 rel err
# end-to-end vs ~9e-7 for FP32; measured on hardware). Every matmul input
# tile is produced by a compute instruction with an FP32R output dtype,
# which the BIR verifier requires (operands must be pre-rounded).
# Set MM_DT = FP32 for full-fp32 matmuls (~4.5x slower on the PE).
MM_DT = FP32R


def _r(ap):
    return ap


def build_kernel(nc, reps=1):
    x_d = nc.declare_dram_parameter("x", [C, L], FP32, isOutput=False)
    gns_d = nc.declare_dram_parameter("gn_scale", [C], FP32, isOutput=False)
    gnb_d = nc.declare_dram_parameter("gn_bias", [C], FP32, isOutput=False)
    wq_d = nc.declare_dram_parameter("wq", [C, C], FP32, isOutput=False)
    bq_d = nc.declare_dram_parameter("bq", [C], FP32, isOutput=False)
    wk_d = nc.declare_dram_parameter("wk", [C, C], FP32, isOutput=False)
    bk_d = nc.declare_dram_parameter("bk", [C], FP32, isOutput=False)
    wv_d = nc.declare_dram_parameter("wv", [C, C], FP32, isOutput=False)
    bv_d = nc.declare_dram_parameter("bv", [C], FP32, isOutput=False)
    wo_d = nc.declare_dram_parameter("wo", [C, C], FP32, isOutput=False)
    bo_d = nc.declare_dram_parameter("bo", [C], FP32, isOutput=False)
    out_d = nc.declare_dram_parameter("out", [C, L], FP32, isOutput=True)

    with tile.TileContext(nc) as tc:
        for _ in range(reps):
            _body(nc, tc, x_d, gns_d, gnb_d, wq_d, bq_d, wk_d, bk_d,
                  wv_d, bv_d, wo_d, bo_d, out_d)
    return nc


def _body(nc, tc, x_d, gns_d, gnb_d, wq_d, bq_d, wk_d, bk_d,
          wv_d, bv_d, wo_d, bo_d, out_d):
    from contextlib import ExitStack

    Id = mybir.ActivationFunctionType.Identity
    Exp = mybir.ActivationFunctionType.Exp
    Sqrt = mybir.ActivationFunctionType.Sqrt
    Square = mybir.ActivationFunctionType.Square
    add = mybir.AluOpType.add
    mult = mybir.AluOpType.mult
    sub = mybir.AluOpType.subtract

    with ExitStack() as ctx:
        consts = ctx.enter_context(tc.tile_pool(name="consts", bufs=1))
        vecs = ctx.enter_context(tc.tile_pool(name="vecs", bufs=1))
        qt_pool = ctx.enter_context(tc.tile_pool(name="qt", bufs=NCT))
        kt_pool = ctx.enter_context(tc.tile_pool(name="kt", bufs=NCT))
        vl_pool = ctx.enter_context(tc.tile_pool(name="vl", bufs=NLT))
        wot_pool = ctx.enter_context(tc.tile_pool(name="wot", bufs=NCT))

        # ---- constants ----
        onesf = consts.tile([128, 128], FP32, tag="onesf")
        nc.vector.memset(onesf[:], 1.0)
        ones = consts.tile([128, 128], MM_DT, tag="ones")
        nc.vector.tensor_copy(ones[:], onesf[:])
        ident = consts.tile([128, 128], FP32, tag="ident")
        nc.vector.memset(ident[:], 1.0)
        # keep where (i - p) == 0
        nc.gpsimd.affine_select(ident[:], ident[:], [[1, 128]],
                                mybir.AluOpType.is_equal, 0.0,
                                base=0, channel_multiplier=-1)
        # group indicator Ind[p, g] = 1 iff p//16 == g   (iota = p - 16 g)
        ind = consts.tile([128, G // 4], FP32, tag="ind")  # [128, 8]
        nc.vector.memset(ind[:], 1.0)
        nc.gpsimd.affine_select(ind[:], ind[:], [[-CPG, G // 4]],
                                mybir.AluOpType.is_ge, 0.0,
                                base=0, channel_multiplier=1)
        nc.gpsimd.affine_select(ind[:], ind[:], [[CPG, G // 4]],
                                mybir.AluOpType.is_ge, 0.0,
                                base=CPG - 1, channel_multiplier=-1)
        # J[g, p] = 1 iff p//16 == g  (iota = p - 16 g)
        jmat = consts.tile([G // 4, 128], FP32, tag="jmat")  # [8, 128]
        nc.vector.memset(jmat[:], 1.0)
        nc.gpsimd.affine_select(jmat[:], jmat[:], [[1, 128]],
                                mybir.AluOpType.is_ge, 0.0,
                                base=0, channel_multiplier=-CPG)
        nc.gpsimd.affine_select(jmat[:], jmat[:], [[-1, 128]],
                                mybir.AluOpType.is_ge, 0.0,
                                base=CPG - 1, channel_multiplier=CPG)

        # ---- per-partition vectors: [512] -> 4 x [128,1] ----
        def load_vec(dram, name):
            t = vecs.tile([128, NCT], FP32, tag=name, name=name + "_sb")
            for i in range(NCT):
                nc.sync.dma_start(out=t[:, i:i + 1],
                                  in_=dram[128 * i:128 * (i + 1)].unsqueeze(-1))
            return t


        qt_t, kt_t, vl_t = [], [], []
        wo_T = []

        with ExitStack() as setup_ctx:
            wsb = setup_ctx.enter_context(tc.tile_pool(name="wsb", bufs=2))
            xp = setup_ctx.enter_context(tc.tile_pool(name="xp", bufs=2))
            hp = setup_ctx.enter_context(tc.tile_pool(name="hp", bufs=NCT))
            gn_sb = setup_ctx.enter_context(tc.tile_pool(name="gnsb", bufs=1))
            wqT_pool = setup_ctx.enter_context(tc.tile_pool(name="wqT", bufs=NCT))
            wkT_pool = setup_ctx.enter_context(tc.tile_pool(name="wkT", bufs=NCT))
            wvT_pool = setup_ctx.enter_context(tc.tile_pool(name="wvT", bufs=NCT))
            ps_t = setup_ctx.enter_context(
                tc.tile_pool(name="ps_t", bufs=2, space="PSUM"))
            ps_gn = setup_ctx.enter_context(
                tc.tile_pool(name="ps_gn", bufs=1, space="PSUM"))
            ps_p = setup_ctx.enter_context(
                tc.tile_pool(name="ps_p", bufs=4, space="PSUM"))

            # ---- transpose the four weights (PE transpose) ----
            for w_d, nm, pool, dst in ((wq_d, "wq", wqT_pool, None),
                                       (wk_d, "wk", wkT_pool, None),
                                       (wv_d, "wv", wvT_pool, None),
                                       (wo_d, "wo", wot_pool, wo_T)):
                wT_tiles = [pool.tile([128, C], MM_DT,
                                      tag=pool.name, name=f"{nm}T{t}")
                            for t in range(NCT)]
                for u in range(NCT):
                    w_t = wsb.tile([128, C], FP32, tag="wsb", name=f"w{nm}{u}")
                    nc.sync.dma_start(out=w_t[:],
                                      in_=w_d[128 * u:128 * (u + 1), :])
                    for t in range(NCT):
                        tp = ps_t.tile([128, 128], FP32, tag="tp", name="tp")
                        nc.tensor.transpose(tp[:],
                                            w_t[:, 128 * t:128 * (t + 1)],
                                            ident[:])
                        nc.vector.tensor_copy(
                            wT_tiles[t][:, 128 * u:128 * (u + 1)], tp[:])
                if nm == "wq":
                    wq_T = wT_tiles
                elif nm == "wk":
                    wk_T = wT_tiles
                elif nm == "wv":
                    wv_T = wT_tiles
                else:
                    wo_T.extend(wT_tiles)

            # small per-partition vectors (issued after the big loads so
            # they don't head-block the DMA queues)
            gns_t = load_vec(gns_d, "gns")
            gnb_t = load_vec(gnb_d, "gnb")
            bq_t = load_vec(bq_d, "bq")
            bk_t = load_vec(bk_d, "bk")
            bo_t = load_vec(bo_d, "bo")

            # ---- load x, compute GN stats ----
            ht = [hp.tile([128, L], MM_DT, tag="hp", name=f"h{t}")
                  for t in range(NCT)]
            stats = gn_sb.tile([128, 2 * NCT], FP32, tag="stats")
            xtiles = {}
            for t in range(NCT):
                x_t = xp.tile([128, L], FP32, tag="xp", name=f"x{t}")
                xtiles[t] = x_t
                nc.sync.dma_start(out=x_t[:], in_=x_d[128 * t:128 * (t + 1), :])
                # per-partition sum and sum of squares (square output is
                # dumped into the h tile, later overwritten by the GN apply)
                nc.vector.tensor_reduce(stats[:, 2 * t:2 * t + 1], x_t[:],
                                        mybir.AxisListType.X, add)
                nc.scalar.activation(ht[t][:], x_t[:], Square,
                                     accum_out=stats[:, 2 * t + 1:2 * t + 2])

            # ---- finish GN stats: group reduce + broadcast ----
            inv_n = float(1.0 / (CPG * L))
            gsum_ps = ps_gn.tile([G // 4, 2 * NCT], FP32, tag="gsum")
            nc.tensor.matmul(gsum_ps[:], ind[:], stats[:])
            # mr[:, 0:4] = mean, mr[:, 4:8] = rstd   per c-tile column
            mr = gn_sb.tile([G // 4, 2 * NCT], FP32, tag="mr")
            tmp8 = gn_sb.tile([G // 4, NCT], FP32, tag="tmp8")
            gview = gsum_ps.rearrange("p (c two) -> p c two", two=2)
            nc.vector.tensor_scalar_mul(mr[:, 0:NCT], gview[:, :, 0], inv_n)
            nc.vector.tensor_scalar_mul(tmp8[:], gview[:, :, 1], inv_n)
            # var = E[x^2] - mean^2 ; rstd = 1/sqrt(var + eps)
            var8 = gn_sb.tile([G // 4, NCT], FP32, tag="var8")
            nc.vector.tensor_tensor(var8[:], mr[:, 0:NCT], mr[:, 0:NCT], mult)
            nc.vector.tensor_tensor(var8[:], tmp8[:], var8[:], sub)
            std8 = gn_sb.tile([G // 4, NCT], FP32, tag="std8")
            eps8 = gn_sb.tile([G // 4, 1], FP32, tag="eps8")
            nc.vector.memset(eps8[:], EPS)
            nc.scalar.activation(std8[:], var8[:], Sqrt, bias=eps8[:])
            nc.vector.reciprocal(mr[:, NCT:2 * NCT], std8[:])

            # broadcast mean/rstd to per-partition, apply GN affine.
            # xp (bufs=2) still holds x2, x3 after the stats pass, so apply
            # those first and only re-DMA x0, x1.
            mr_v = mr.rearrange("p (h f) -> p h f", h=2)
            APPLY_ORDER = [2, 3, 0, 1]
            for t in APPLY_ORDER:
                bc = ps_t.tile([128, 2], FP32, tag="tp", name=f"bc{t}")
                nc.tensor.matmul(bc[:], jmat[:], mr_v[:, :, t])
                a_t = gn_sb.tile([128, 1], FP32, tag=f"a{t}", name=f"a{t}")
                b_t = gn_sb.tile([128, 1], FP32, tag=f"b{t}", name=f"b{t}")
                nc.vector.tensor_tensor(a_t[:], bc[:, 1:2], gns_t[:, t:t + 1],
                                        mult)
                # b = gn_bias - mean * a
                nc.vector.tensor_tensor(b_t[:], bc[:, 0:1], a_t[:], mult)
                nc.vector.tensor_tensor(b_t[:], gnb_t[:, t:t + 1], b_t[:], sub)
                if t in (2, 3):
                    x_src = xtiles[t]
                else:
                    x_src = xp.tile([128, L], FP32, tag="xp", name=f"xb{t}")
                    nc.sync.dma_start(out=x_src[:],
                                      in_=x_d[128 * t:128 * (t + 1), :])
                nc.scalar.activation(ht[t][:], x_src[:], Id,
                                     bias=b_t[:], scale=a_t[:])

            # ---- bv broadcast [128, C] via replicated DMA ----
            bvb = gn_sb.tile([128, C], FP32, tag="bvb")
            nc.sync.dma_start(out=bvb[:],
                              in_=bv_d[:].unsqueeze(0).partition_broadcast(128))

            # ---- projections ----
            for dst_list, wT_l, bvec, pool, nmo in (
                    (qt_t, None, bq_t, qt_pool, "q"),
                    (kt_t, None, bk_t, kt_pool, "k")):
                wT_l = wq_T if nmo == "q" else wk_T
                for t in range(NCT):
                    dtile = pool.tile([128, L], MM_DT, tag=pool.name,
                                      name=f"{nmo}T{t}")
                    for lc in range(NQC):
                        pp = ps_p.tile([128, 512], FP32, tag="pp", name="pp")
                        for j, ci in enumerate([2, 3, 0, 1]):
                            nc.tensor.matmul(
                                pp[:],
                                _r(wT_l[ci][:, 128 * t:128 * (t + 1)]),
                                _r(ht[ci][:, 512 * lc:512 * (lc + 1)]),
                                start=(j == 0), stop=(j == NCT - 1))
                        nc.scalar.activation(dtile[:, 512 * lc:512 * (lc + 1)],
                                             pp[:], Id, bias=bvec[:, t:t + 1])
                    dst_list.append(dtile)

            for lt in range(NLT):
                vtile = vl_pool.tile([128, C], MM_DT, tag="vl",
                                     name=f"vl{lt}")
                pp = ps_p.tile([128, 512], FP32, tag="pp", name="pp")
                for j, ci in enumerate([2, 3, 0, 1]):
                    nc.tensor.matmul(
                        pp[:],
                        _r(ht[ci][:, 128 * lt:128 * (lt + 1)]),
                        _r(wv_T[ci][:]),
                        start=(j == 0), stop=(j == NCT - 1))
                nc.vector.tensor_tensor(vtile[:], pp[:], bvb[:], add)
                vl_t.append(vtile)

        # ---- attention ----
        with ExitStack() as att_ctx:
            pt_pool = att_ctx.enter_context(tc.tile_pool(name="pt", bufs=3))
            dinv_pool = att_ctx.enter_context(tc.tile_pool(name="dinv", bufs=2))
            asb_pool = att_ctx.enter_context(tc.tile_pool(name="asb", bufs=8))
            osb_pool = att_ctx.enter_context(tc.tile_pool(name="osb", bufs=3))
            xr_pool = att_ctx.enter_context(tc.tile_pool(name="xr", bufs=3))
            ps_s = att_ctx.enter_context(
                tc.tile_pool(name="ps_s", bufs=2, space="PSUM"))
            ps_a = att_ctx.enter_context(
                tc.tile_pool(name="ps_a", bufs=NCT, space="PSUM"))
            ps_d = att_ctx.enter_context(
                tc.tile_pool(name="ps_d", bufs=1, space="PSUM"))
            ps_o = att_ctx.enter_context(
                tc.tile_pool(name="ps_o", bufs=1, space="PSUM"))

            def s_block(qc_i, kt_i):
                s_tile = ps_s.tile([128, 512], FP32, tag="s",
                                   name=f"s{qc_i}_{kt_i}")
                for ci in range(NCT):
                    nc.tensor.matmul(
                        s_tile[:],
                        _r(kt_t[ci][:, 128 * kt_i:128 * (kt_i + 1)]),
                        _r(qt_t[ci][:, 512 * qc_i:512 * qc_i + 512]),
                        start=(ci == 0), stop=(ci == NCT - 1))
                return s_tile

            s_next = s_block(0, 0)
            for qc in range(NQC):
                q0 = 512 * qc
                a_ps = [ps_a.tile([128, 512], FP32, tag="a", name=f"a_ps{cc}")
                        for cc in range(NCT)]
                d_ps = ps_d.tile([128, 512], FP32, tag="d", name="d_ps")

                s_cur = s_next
                for kt_i in range(NLT):
                    p_t = pt_pool.tile([128, 512], MM_DT, tag="pt",
                                       name="p_t")
                    nc.scalar.activation(p_t[:], s_cur[:], Exp, scale=SCALE)
                    if kt_i + 1 < NLT:
                        s_cur = s_block(qc, kt_i + 1)
                    elif qc + 1 < NQC:
                        # prefetch the next q-chunk's first S block so the PE
                        # has work while DVE finishes this chunk's division
                        s_next = s_block(qc + 1, 0)
                    first = kt_i == 0
                    last = kt_i == NLT - 1
                    for cc in range(NCT):
                        nc.tensor.matmul(
                            a_ps[cc][:],
                            _r(vl_t[kt_i][:, 128 * cc:128 * (cc + 1)]),
                            _r(p_t[:]),
                            start=first, stop=last, skip_group_check=True)
                    # softmax denominator: ones-matmul accumulation (the
                    # result lands broadcast across all 128 partitions)
                    nc.tensor.matmul(d_ps[:], _r(ones[:]), _r(p_t[:]),
                                     start=first, stop=last,
                                     skip_group_check=True)

                # Evacuate UNNORMALIZED A via plain ACT copies (frees the
                # accumulation banks without waiting for the reciprocal);
                # the softmax division commutes with the out-projection
                # (it is a per-column scaling), so it is applied at the
                # final evacuation instead.
                a_sb = []
                for cc in range(NCT):
                    t = asb_pool.tile([128, 512], MM_DT, tag="asb",
                                      name=f"asb{cc}")
                    nc.scalar.copy(t[:], a_ps[cc][:])
                    a_sb.append(t)
                dinv = dinv_pool.tile([128, 512], FP32, tag="dinv", name="dinv")
                dscr = dinv_pool.tile([128, 512], FP32, tag="dscr", name="dscr")
                nc.vector.reciprocal_approx_accurate(out=dinv[:], in_=d_ps[:],
                                                     scratch=dscr[:])

                for ot in range(NCT):
                    o_ps = ps_o.tile([128, 512], FP32, tag="o", name="o_ps")
                    for cc in range(NCT):
                        nc.tensor.matmul(
                            o_ps[:],
                            _r(wo_T[cc][:, 128 * ot:128 * (ot + 1)]),
                            _r(a_sb[cc][:]),
                            start=(cc == 0), stop=(cc == NCT - 1))
                    xr = xr_pool.tile([128, 512], FP32, tag="xr", name="xr")
                    nc.sync.dma_start(
                        out=xr[:],
                        in_=x_d[128 * ot:128 * (ot + 1), q0:q0 + 512])
                    tmp = osb_pool.tile([128, 512], FP32, tag="otmp",
                                        name="otmp")
                    nc.vector.tensor_tensor(tmp[:], o_ps[:], dinv[:], mult)
                    osb = osb_pool.tile([128, 512], FP32, tag="osb", name="osb")
                    nc.vector.scalar_tensor_tensor(
                        osb[:], tmp[:], bo_t[:, ot:ot + 1], xr[:],
                        op0=add, op1=add)
                    nc.sync.dma_start(
                        out=out_d[128 * ot:128 * (ot + 1), q0:q0 + 512],
                        in_=osb[:])


def make_nc():
    return bacc.Bacc("TRN2", target_bir_lowering=False, debug=False)


_NC_CACHE = []


def kernel(**inputs):
    from concourse.bass_utils import run_bass_kernel_spmd

    x = np.ascontiguousarray(inputs["x"], dtype=np.float32)
    B = x.shape[0]
    assert B == 8, f"kernel is built for B=8 (one batch element per core), got {B}"
    shared = {}
    for name in ("gn_scale", "gn_bias", "wq", "bq", "wk", "bk",
                 "wv", "bv", "wo", "bo"):
        shared[name] = np.ascontiguousarray(inputs[name], dtype=np.float32)

    if not _NC_CACHE:
        nc = make_nc()
        build_kernel(nc)
        nc.compile()
        _NC_CACHE.append(nc)
    nc = _NC_CACHE[0]

    core_ids = list(range(B))
    in_maps = [dict(shared, x=x[i]) for i in range(B)]
    res = run_bass_kernel_spmd(nc, in_maps, core_ids)
    out = np.stack([res.results[i]["out"] for i in range(B)], axis=0)
    return out.astype(np.float32)


if __name__ == "__main__":
    rng = np.random.default_rng(0)
    demo = {
        "x": rng.standard_normal((8, C, L), dtype=np.float32),
        "gn_scale": np.ones(C, np.float32),
        "gn_bias": np.zeros(C, np.float32),
    }
    for w, b in (("wq", "bq"), ("wk", "bk"), ("wv", "bv"), ("wo", "bo")):
        demo[w] = rng.standard_normal((C, C), dtype=np.float32) / np.sqrt(C)
        demo[b] = np.zeros(C, np.float32)
    out = kernel(**demo)
    print(out.shape, out.dtype)
